# revision 30
# baseline (speedup 1.0000x reference)
"""Trainium2 Bass kernel for nn_AudioLSTM: 2-layer bidirectional LSTM.

Reference computation (PyTorch gate order i,f,g,o):
  layer0: BiLSTM(x[B,T,80]) -> out0[B,T,256]
  layer1: BiLSTM(out0)      -> final hidden [B, 256] = cat(h_fwd_last, h_bwd_last)

Strategy:
  - Data-parallel over batch: 8 cores x 8 batch. Each core runs both
    directions of both layers for its batch slice.
  - State layout [H=128 partitions, batch=8 free]; gate pre-activations per
    step are [128, 32] (4 gate slots x 8 batch) accumulated in PSUM.
  - Input contributions (x @ WiT + biases) are matmul'd just-in-time into the
    same PSUM region the recurrence matmuls accumulate onto (start=True from
    the JIT pass, start=False accumulate from the recurrence matmuls).
  - All four gates go through ONE tanh activation per step:
    sigmoid(z) = (tanh(z/2)+1)/2 with the halving folded into the i,f,o
    weight columns. The cell state is tracked as X = 2c and h as H2 = 2h
    (weights consuming h are pre-halved; the host halves the final output),
    which lets the whole cell update run in 3 fused DVE ops:
      q  = (in0 + 1) * in1   with in0=[Ti,Tf], in1=[Tg, X_prev]
                             ->  q = [2*i*g, 2*sig_f*X_prev]
      X' = q1*0.5 + q0       (= 2*c_new)
      H2 = (To + 1) * tanh(0.5*X')   (tanh's free input scale)
    Per step per direction: 4 matmuls, 2 ACT ops, 3 DVE ops — one fewer DVE
    hop on the recurrence critical chain than the sigmoid formulation.
  - The per-(layer,dir) CS tile packs the gate tanh outputs S and both cell
    ping-pong slots X0/X1 so q's second operand is a single strided AP.
  - The backward direction is the same code on time-reversed data; reversed
    access uses negative-stride APs. Backward layer-0 outputs are stored at
    original time positions so layer-1 forward reads everything contiguously.
"""

import sys

if "/opt/trn_rl_repo" not in sys.path:
    sys.path.insert(0, "/opt/trn_rl_repo")

import os as _os0
import numpy as np

import concourse.bacc as bacc
import concourse.bass as bass
import concourse.mybir as mybir
import concourse.tile as tile

F32 = mybir.dt.float32
BF16 = mybir.dt.bfloat16

B, T, DIN, H = 64, 1500, 80, 128
NCORES = 8
BLOC = B // NCORES          # batch per core
CHUNK = int(_os0.environ.get("LSTM_CHUNK", "12"))  # steps per PSUM chunk; divides T
NCH = T // CHUNK
SLAB_CH = 25                # x-slab size in chunks
RING = 4                    # layer-1 h ring slots

# gate slot order in PSUM/weights: [f, i, g, o]; rows in torch order i,f,g,o
# All four gates go through ONE tanh ACT per step: sigmoid(z)=(tanh(z/2)+1)/2
# with the halving folded into the f,i,o weight columns. h is tracked as
# H2=2h (weights consuming h pre-halved; host divides the output by 2).
SLOT_ROWS = [0, 1, 2, 3]    # slots [i, f, g, o]
SLOT_SCALE = [0.5, 0.5, 1.0, 0.5]

import os as _os

if _os.environ.get("LSTM_WDT", "bf16") == "bf16":
    # matmul operand dtype (weights / x / h). Cell state, gate activations and
    # the final output stay fp32. Validated: rel err ~2.5e-3 at T=1500.
    import ml_dtypes as _mld

    WDT = BF16
    WNP = _mld.bfloat16
else:
    WDT = F32
    WNP = np.float32


def _prep_whT(Whh):
    """Whh [2, 4H, H] -> [128, 1024] stationary: col d*512 + s*128 + j."""
    out = np.empty((H, 2 * 4 * H), dtype=WNP)
    for d in range(2):
        for s in range(4):
            blk = Whh[d, SLOT_ROWS[s] * H:(SLOT_ROWS[s] + 1) * H, :]  # [128, H]
            out[:, d * 512 + s * 128: d * 512 + (s + 1) * 128] = (
                0.5 * SLOT_SCALE[s] * blk.T)
    return out


def _prep_wiT0(Wih, bih, bhh):
    """[2,4H,80]+biases -> [81, 1024]; row 80 is the bias row."""
    out = np.empty((DIN + 1, 2 * 4 * H), dtype=WNP)
    bias = bih + bhh
    for d in range(2):
        for s in range(4):
            r0 = SLOT_ROWS[s] * H
            cols = slice(d * 512 + s * 128, d * 512 + (s + 1) * 128)
            out[:DIN, cols] = SLOT_SCALE[s] * Wih[d, r0:r0 + H, :].T
            out[DIN, cols] = SLOT_SCALE[s] * bias[d, r0:r0 + H]
    return out


def _prep_wiT1(Wih, half):
    """Wih1 [2, 4H, 256] half (0:fwd-feat, 1:bwd-feat) -> [128, 1024]."""
    out = np.empty((H, 2 * 4 * H), dtype=WNP)
    for d in range(2):
        for s in range(4):
            r0 = SLOT_ROWS[s] * H
            blk = Wih[d, r0:r0 + H, half * H:(half + 1) * H]
            out[:, d * 512 + s * 128: d * 512 + (s + 1) * 128] = (
                0.5 * SLOT_SCALE[s] * blk.T)
    return out


def _prep_b1(bih, bhh):
    out = np.empty((1, 2 * 4 * H), dtype=WNP)
    bias = bih + bhh
    for d in range(2):
        for s in range(4):
            r0 = SLOT_ROWS[s] * H
            out[0, d * 512 + s * 128: d * 512 + (s + 1) * 128] = (
                SLOT_SCALE[s] * bias[d, r0:r0 + H])
    return out


def _prep_x(x_core, tt=T):
    """x [BLOC, tt, 80] -> [81, tt*BLOC] with col t*BLOC+b; row 80 = ones."""
    out = np.empty((DIN + 1, tt * BLOC), dtype=WNP)
    # [tt, BLOC, DIN] -> transpose to [DIN, tt, BLOC]
    out[:DIN] = np.ascontiguousarray(x_core.transpose(2, 1, 0)).reshape(
        DIN, tt * BLOC)
    out[DIN] = 1.0
    return out


def build_nc(tt=T):
    """Emit the Bass program for sequence length tt (must divide by CHUNK)."""
    nch = tt // CHUNK
    nc = bacc.Bacc("TRN2", target_bir_lowering=False, debug=False)

    x_in = nc.declare_dram_parameter("x", [DIN + 1, tt * BLOC], WDT,
                                     isOutput=False)
    wh0_in = nc.declare_dram_parameter("wh0", [H, 1024], WDT, isOutput=False)
    wi0_in = nc.declare_dram_parameter("wi0", [DIN + 1, 1024], WDT,
                                       isOutput=False)
    wh1_in = nc.declare_dram_parameter("wh1", [H, 1024], WDT, isOutput=False)
    wi1f_in = nc.declare_dram_parameter("wi1f", [H, 1024], WDT, isOutput=False)
    wi1b_in = nc.declare_dram_parameter("wi1b", [H, 1024], WDT, isOutput=False)
    b1_in = nc.declare_dram_parameter("b1", [1, 1024], WDT, isOutput=False)
    hout = nc.declare_dram_parameter("hout", [2, H, BLOC], F32, isOutput=True)

    with tile.TileContext(nc) as tc:
        _emit(nc, tc, tt, nch, x_in, wh0_in, wi0_in, wh1_in, wi1f_in, wi1b_in,
              b1_in, hout)
    nc.compile()
    if _os.environ.get("LSTM_LDWFIX", "1") == "1":
        _retarget_ldw_waits(nc)
    if _os.environ.get("LSTM_EVSFIX", "1") == "1":
        _elide_act_eventsems(nc)
    return nc


def _elide_act_eventsems(nc):
    """Fold single-wait EventSemaphores into the following Activation.

    bacc emits `EventSemaphore(wait=W); Activation(wait=own-engine-sem)`
    because an instruction holds one wait. The own-engine wait is trivially
    satisfied (engines execute in order), so the Activation can carry W
    directly and the EventSemaphore dispatch disappears.
    """
    import concourse.mybir as mb
    for blk in nc.m.functions[0].blocks:
        insts = blk.instructions
        drop = []
        for i in range(len(insts) - 1):
            ev, act = insts[i], insts[i + 1]
            if (type(ev).__name__ != "InstEventSemaphore"
                    or type(act).__name__ != "InstActivation"):
                continue
            esi, asi = ev.sync_info, act.sync_info
            ew = list(esi.on_wait) if esi and esi.on_wait else []
            eu = list(esi.on_update) if esi and esi.on_update else []
            aw = list(asi.on_wait) if asi and asi.on_wait else []
            if len(ew) != 1 or eu:
                continue
            if len(aw) != 1 or not (aw[0].ant_name or "").startswith(
                    "Activation"):
                continue
            if getattr(ev, "engine", None) != getattr(act, "engine", None):
                continue
            asi.on_wait = ew
            drop.append(i)
        for i in reversed(drop):
            del insts[i]


def _retarget_ldw_waits(nc):
    """Move compute-engine waits off LDWEIGHTS onto the following MATMUL.

    bacc's move_matmul_waits_to_ldweights leaves the h-dependency (DVE) wait
    on the weight load, putting the load itself on the recurrence critical
    ring. LDWEIGHTS only reads constant weight tiles (DMA-written at start),
    never DVE/ACT-written tiles, and the PE executes in order, so swapping the
    wait assignments between an LDWEIGHTS and its immediately-following MATMUL
    preserves every true ordering edge while letting the weight load run
    early. Only compute-engine sems (DVE/Activation/Pool) are touched; DMA
    waits stay put.
    """
    import concourse.mybir as mb
    movable = ("DVE", "Activation", "Pool")
    for blk in nc.m.functions[0].blocks:
        insts = blk.instructions
        for i in range(len(insts) - 1):
            ldw, mm = insts[i], insts[i + 1]
            if (type(ldw).__name__ != "InstLdweights"
                    or type(mm).__name__ != "InstMatmult"):
                continue
            lsi, msi = ldw.sync_info, mm.sync_info
            lw = list(lsi.on_wait) if lsi and lsi.on_wait else []
            if not lw or not all(
                    (w.ant_name or "").startswith(movable) for w in lw):
                continue
            mw = list(msi.on_wait) if msi and msi.on_wait else []
            if (all((w.ant_name or "").startswith("Activation") for w in lw)
                    and any((w.ant_name or "").startswith("DVE")
                            for w in mw)):
                # The Activation wait is the PSUM WAR edge vs the sigmoid
                # read; the matmul's DVE wait (h-write) transitively implies
                # it (hmult waits the tanh which follows that sigmoid on the
                # in-order ACT engine), so drop it and let the weight load
                # run entirely inside the idle window. (HW-validated.)
                lsi.on_wait = []
                continue
            if len(mw) + 0 > 1:
                continue
            # swap: LDW gets MM's waits (possibly none), MM gets LDW's
            if lsi is None:
                continue
            if msi is None:
                mm.sync_info = mb.SyncInfo(on_wait=[], on_update=[])
                msi = mm.sync_info
            lsi.on_wait = mw
            msi.on_wait = lw


def _emit(nc, tc, tt, nch, x_in, wh0_in, wi0_in, wh1_in, wi1f_in, wi1b_in,
          b1_in, hout):
    from contextlib import ExitStack
    ctx = ExitStack()
    const = ctx.enter_context(tc.tile_pool(name="const", bufs=1))
    spool = ctx.enter_context(tc.tile_pool(name="spool", bufs=int(_os.environ.get("LSTM_SBUFS", "6"))))
    mpool = ctx.enter_context(tc.tile_pool(name="mpool", bufs=int(_os.environ.get("LSTM_MBUFS", "6"))))
    ppool = ctx.enter_context(tc.tile_pool(
        name="ppool", bufs=int(_os.environ.get("LSTM_PBUFS", "4")),
        space="PSUM"))

    # ---- persistent tiles ----
    wh0 = const.tile([H, 1024], WDT, tag="wh0", name="wh0")
    wi0 = const.tile([DIN + 1, 1024], WDT, tag="wi0", name="wi0")
    wh1 = const.tile([H, 1024], WDT, tag="wh1", name="wh1")
    wi1f = const.tile([H, 1024], WDT, tag="wi1f", name="wi1f")
    wi1b = const.tile([H, 1024], WDT, tag="wi1b", name="wi1b")
    b1 = const.tile([1, 1024], WDT, tag="b1", name="b1")
    ones = const.tile([1, CHUNK * BLOC], WDT, tag="ones", name="ones")
    z8 = const.tile([H, BLOC], WDT, tag="z8", name="z8")

    nslab = (nch + SLAB_CH - 1) // SLAB_CH
    xsl = [const.tile([DIN + 1, min(SLAB_CH, nch - i * SLAB_CH) * CHUNK * BLOC],
                      WDT, tag=f"xsl{i}", name=f"xsl{i}") for i in range(nslab)]
    out0 = [[const.tile([H, CHUNK * BLOC], WDT, tag=f"out0_{d}_{c}", name=f"out0_{d}_{c}")
             for c in range(nch)] for d in range(2)]
    hring = [const.tile([H, RING * BLOC], WDT, tag=f"hring{d}", name=f"hring{d}")
             for d in range(2)]
    hfin = [const.tile([H, BLOC], F32, tag=f"hfin{d}", name=f"hfin{d}") for d in range(2)]
    # Combined per-(layer,dir) state tile so the fused q-op can read the
    # cell state X=2c and the gate tanh Tg through ONE strided AP:
    #   cols [0:32]  = S slot0 (gates i,f,g,o of even steps, 8 cols each)
    #   cols [32:40] = X slot0 (even steps)   [40:48] = X slot1 (odd steps)
    #   cols [48:80] = S slot1 (odd steps)
    CS = [[const.tile([H, 80], F32, tag=f"cs{l}{d}", name=f"cs{l}{d}")
           for d in range(2)] for l in range(2)]

    def cs_S(layer, d, k):
        s = k % 2
        return CS[layer][d][:, (0 if s == 0 else 48):(32 if s == 0 else 80)]

    def cs_S_col(layer, d, k):
        return 0 if k % 2 == 0 else 48

    def cs_X_col(k):
        return 32 if k % 2 == 0 else 40

    def two_blocks(tilev, col0, col1):
        """AP [p, 2, 8]: 8-wide blocks at col0 then col1 (cols % 8 == 0)."""
        v = tilev.rearrange("p (a c) -> p a c", c=8)
        a0, a1 = col0 // 8, col1 // 8
        step = a1 - a0
        stop = a1 + 1 if step > 0 else a1 - 1
        assert stop >= -1 and (step > 0 or a1 >= 1)
        return v[:, a0:stop:step, :]

    # ---- loads / inits ----
    nc.sync.dma_start(out=wh0[:], in_=wh0_in[:])
    nc.sync.dma_start(out=wi0[:], in_=wi0_in[:])
    nc.sync.dma_start(out=wh1[:], in_=wh1_in[:])
    nc.sync.dma_start(out=wi1f[:], in_=wi1f_in[:])
    nc.sync.dma_start(out=wi1b[:], in_=wi1b_in[:])
    nc.sync.dma_start(out=b1[:], in_=b1_in[:])
    col0 = 0
    for i, xs in enumerate(xsl):
        w = xs.shape[1]
        nc.sync.dma_start(out=xs[:], in_=x_in[:, col0:col0 + w])
        col0 += w
    nc.vector.memset(ones[:], 1.0)
    nc.vector.memset(z8[:], 0.0)

    Sig = mybir.ActivationFunctionType.Sigmoid
    Tanh = mybir.ActivationFunctionType.Tanh
    MUL = mybir.AluOpType.mult
    ADD = mybir.AluOpType.add
    SUB = mybir.AluOpType.subtract

    def xsl_chunk(c, rev):
        """moving operand [81, 96] for layer-0 chunk c (processing order)."""
        if not rev:
            sl, off = c // SLAB_CH, (c % SLAB_CH) * CHUNK * BLOC
            return xsl[sl][:, off:off + CHUNK * BLOC]
        c2 = (nch - 1) - c
        sl, off = c2 // SLAB_CH, (c2 % SLAB_CH) * CHUNK * BLOC
        v = xsl[sl][:, off:off + CHUNK * BLOC]
        return v.rearrange("p (s b) -> p s b", b=BLOC)[:, ::-1, :]

    def out0_chunk(dsrc, c, rev):
        """moving operand [128, 96] from layer-0 outputs (original-time order)."""
        if not rev:
            return out0[dsrc][c][:, :]
        c2 = (nch - 1) - c
        v = out0[dsrc][c2][:, :]
        return v.rearrange("p (s b) -> p s b", b=BLOC)[:, ::-1, :]

    def jit_mms(layer, d, c, pt):
        """Input-contribution matmuls for chunk c of (layer, dir d) -> list."""
        # NOTE: start=True marks the whole 2KB PSUM bank "pending zero", so it
        # must appear on exactly the FIRST matmul touching the bank each round;
        # all later matmuls (incl. the recurrence ones) then overwrite-once /
        # accumulate per the per-byte pending state.
        mms = []
        for s in range(4):
            dst = pt[:, s * CHUNK * BLOC:(s + 1) * CHUNK * BLOC]
            wcol = slice(d * 512 + s * 128, d * 512 + (s + 1) * 128)
            if layer == 0:
                mms.append((dst, wi0[:, wcol], xsl_chunk(c, d == 1), s == 0))
            else:
                mms.append((dst, wi1f[:, wcol], out0_chunk(0, c, d == 1), s == 0))
                mms.append((dst, wi1b[:, wcol], out0_chunk(1, c, d == 1), False))
                mms.append((dst, b1[:, wcol], ones[:], False))
        return mms

    def emit_jit(mm):
        dst, lhsT, rhs, start = mm
        nc.tensor.matmul(dst, lhsT, rhs, start=start, stop=False,
                         skip_group_check=True)

    def h_prev(layer, d, k):
        if k == 0:
            return z8[:]
        if layer == 0:
            t = k - 1 if d == 0 else tt - k
            return out0[d][t // CHUNK][:, (t % CHUNK) * BLOC:
                                       (t % CHUNK + 1) * BLOC]
        s = (k - 1) % RING
        return hring[d][:, s * BLOC:(s + 1) * BLOC]

    def h_dst(layer, d, k):
        if layer == 0:
            t = k if d == 0 else tt - 1 - k
            return out0[d][t // CHUNK][:, (t % CHUNK) * BLOC:
                                       (t % CHUNK + 1) * BLOC]
        if k == tt - 1:
            return hfin[d][:]
        s = k % RING
        return hring[d][:, s * BLOC:(s + 1) * BLOC]

    GPS_M1 = _os.environ.get("LSTM_GPS_M1", "0") == "1"
    IL_DVE = _os.environ.get("LSTM_IL_DVE", "0") == "1"
    SPLIT_SIG = _os.environ.get("LSTM_SPLIT_SIG", "0") == "1"
    # timing-only ablations (break numerics): norec = skip recurrence matmuls,
    # noact = replace sigmoid/tanh with DVE copies, nodve = skip c-path DVE
    ABL = _os.environ.get("LSTM_ABLATE", "")

    def step_mms(layer, d, k, pt, wh):
        if ABL == "norec":
            return
        sk = k % CHUNK
        hp = h_prev(layer, d, k)
        for s in range(4):
            dst = pt[:, s * CHUNK * BLOC + sk * BLOC:
                     s * CHUNK * BLOC + (sk + 1) * BLOC]
            nc.tensor.matmul(dst, wh[:, d * 512 + s * 128:d * 512 + (s + 1) * 128],
                             hp, start=False,
                             stop=(sk == CHUNK - 1 and s == 3),
                             skip_group_check=True)

    def step_act(layer, d, k, pt):
        """Gate tanh -> S region of the CS tile, split in two: [i,f,g] on the
        recurrence critical chain (the q-op needs them), o in a second ACT
        that runs in the chain's idle window (only h2 needs To, much later).
        Slots: [i, f, g, o]; i,f,o pre-acts arrive halved (weight prep), so
        S = [Ti, Tf, Tg, To] with sigmoid(z) = (T(z/2)+1)/2 recoverable."""
        sk = k % CHUNK
        view = pt.rearrange("p (g s b) -> p g s b", s=CHUNK, b=BLOC)
        nc.scalar.activation(cs_S(layer, d, k), view[:, :, sk, :], Tanh)

    def step_q(layer, d, k):
        """Fused gate products: q = (in0 + 1) * in1 with
        in0 = [Ti, Tf], in1 = [Tg, X_prev]  ->  q = [2*i*g, 2*sigf*X_prev]."""
        cs = CS[layer][d]
        s0 = cs_S_col(layer, d, k)
        q = mpool.tile([H, 2 * BLOC], F32, tag=f"q{d}", name="q")
        in0 = cs[:, s0:s0 + 2 * BLOC].rearrange("p (a c) -> p a c", c=BLOC)
        in1 = two_blocks(cs, s0 + 2 * BLOC, cs_X_col(k - 1))
        qv = q.rearrange("p (a c) -> p a c", c=BLOC)
        nc.vector.scalar_tensor_tensor(qv, in0, 1.0, in1, ADD, MUL)
        return q

    def step_X(layer, d, k, q):
        """X_new = 2c_new = q1*0.5 + q0  (= 2*sigf*c_prev + 2*i*g)."""
        xo = CS[layer][d][:, cs_X_col(k):cs_X_col(k) + BLOC]
        nc.vector.scalar_tensor_tensor(xo, q[:, BLOC:2 * BLOC], 0.5,
                                       q[:, 0:BLOC], MUL, ADD)
        return xo

    def step_tanh(layer, d, k, xo):
        tcl = mpool.tile([H, BLOC], F32, tag=f"tc{d}", name="tc")
        nc.scalar.activation(tcl[:], xo, Tanh, scale=0.5)
        return tcl

    def step_h(layer, d, k, tcl):
        """H2 = (To + 1) * tanh(c) = 2h; weights consuming h are pre-halved,
        the host halves the final output."""
        cs = CS[layer][d]
        s0 = cs_S_col(layer, d, k)
        nc.vector.scalar_tensor_tensor(h_dst(layer, d, k),
                                       cs[:, s0 + 3 * BLOC:s0 + 4 * BLOC],
                                       1.0, tcl[:], ADD, MUL)

    REPS = int(_os.environ.get("LSTM_REPS", "1"))  # timing: repeat whole pass
    for rep in range(REPS):
      for l in range(2):
        for d in range(2):
            nc.vector.memset(CS[l][d][:, 32:48], 0.0)  # zero both X slots
      for layer, wh in ((0, wh0), (1, wh1)):
        npre = 4 if layer == 0 else 12
        pts = {}
        for d in range(2):
            pts[(d, 0)] = ppool.tile([H, 4 * CHUNK * BLOC], F32, tag="pt", name="pt")
            for mm in jit_mms(layer, d, 0, pts[(d, 0)]):
                emit_jit(mm)
        for c in range(nch):
            nxt = [[], []]
            if c + 1 < nch:
                for d in range(2):
                    pts[(d, c + 1)] = ppool.tile([H, 4 * CHUNK * BLOC], F32,
                                                 tag="pt", name="pt")
                    nxt[d] = jit_mms(layer, d, c + 1, pts[(d, c + 1)])
            for sk in range(CHUNK):
                k = c * CHUNK + sk
                # STAGGERED emission: each direction's FULL step block
                # (matmuls -> sigmoid -> c-path -> tanh -> h-write) is
                # emitted contiguously, so the two chains settle at a
                # half-cycle offset. Every engine then sees its stream in
                # ready-time order (sigA tanhA sigB tanhB on ACT rather than
                # sigA sigB tanhA tanhB), eliminating in-order queue stalls
                # between the chains.
                for d in range(2):
                    step_mms(layer, d, k, pts[(d, c)], wh)
                    lo = sk * npre // CHUNK
                    hi = (sk + 1) * npre // CHUNK
                    for mm in nxt[d][lo:hi]:
                        emit_jit(mm)
                    step_act(layer, d, k, pts[(d, c)])
                    q = step_q(layer, d, k)
                    xo = step_X(layer, d, k, q)
                    tcl = step_tanh(layer, d, k, xo)
                    step_h(layer, d, k, tcl)
            for d in range(2):
                del pts[(d, c)]

    nc.sync.dma_start(out=hout[0], in_=hfin[0][:])
    nc.sync.dma_start(out=hout[1], in_=hfin[1][:])
    ctx.close()


def prep_inputs(x, Wih0, Whh0, bih0, bhh0, Wih1, Whh1, bih1, bhh1, tt=T):
    """Full numpy inputs -> list of per-core input maps."""
    x = np.asarray(x, np.float32)
    w = {
        "wh0": _prep_whT(np.asarray(Whh0, np.float32)),
        "wi0": _prep_wiT0(np.asarray(Wih0, np.float32),
                          np.asarray(bih0, np.float32),
                          np.asarray(bhh0, np.float32)),
        "wh1": _prep_whT(np.asarray(Whh1, np.float32)),
        "wi1f": _prep_wiT1(np.asarray(Wih1, np.float32), 0),
        "wi1b": _prep_wiT1(np.asarray(Wih1, np.float32), 1),
        "b1": _prep_b1(np.asarray(bih1, np.float32),
                       np.asarray(bhh1, np.float32)),
    }
    maps = []
    for core in range(NCORES):
        xs = _prep_x(x[core * BLOC:(core + 1) * BLOC, :tt], tt)
        maps.append({"x": xs, **w})
    return maps


def assemble_out(results):
    """Per-core hout [2, 128, 8] -> [64, 256] float32."""
    out = np.empty((B, 2 * H), np.float32)
    for core, res in enumerate(results):
        ho = res["hout"]
        for b in range(BLOC):
            out[core * BLOC + b, :H] = 0.5 * ho[0, :, b]
            out[core * BLOC + b, H:] = 0.5 * ho[1, :, b]
    return out


_NC_CACHE = {}


def kernel(x, Wih0, Whh0, bih0, bhh0, Wih1, Whh1, bih1, bhh1):
    from concourse.bass_utils import run_bass_kernel_spmd

    if T not in _NC_CACHE:
        _NC_CACHE[T] = build_nc(T)
    nc = _NC_CACHE[T]
    maps = prep_inputs(x, Wih0, Whh0, bih0, bhh0, Wih1, Whh1, bih1, bhh1)
    res = run_bass_kernel_spmd(nc, maps, list(range(NCORES)))
    return assemble_out(res.results)

